# revision 4
# baseline (speedup 1.0000x reference)
"""TRN2 Bass kernel for nn_MFNet (scatter_memory, memory regime).

reference:
    pred  = embed_user @ embed_item.T          [4096, 16384]
    mask  = zeros.at[rows, item_sets].set(1)
    label = zeros.at[rows, item_sets].set(rating_sets)
    return pred * mask, label, sparsity

pred*mask is ~99.9% zeros, so the dense matmul is never materialized.
Users are sharded contiguously across 8 cores (rows [c*512,(c+1)*512));
entries are routed on the host to the owning core and deduplicated
(last write wins, matching XLA scatter update order).

Device kernel per core (raw bass, three engine queues):
  DVE : memset zero tile; dots[e] = sum_h Ug[e,h]*Ig[e,h] (gathered rows
        are host-prepared inputs); pred chunk payloads = dots * one-hot
        mask (host input) so each 64-elem chunk holds its dot at the
        right position.
  SP  : loads; zero-fills both outputs [512,16384] with 8 block DMAs
        [128,16384] from the zero tile (per-block semaphores).
  Pool: per 128-row block, right after its zero DMA lands: indirect
        chunk scatters (128 offsets/op, 64 f32 contiguous payload per
        offset -- the HW's native indirect mode) for label then pred;
        finally single-element fixup scatters for pred chunk collisions.

Padding writes zero payloads to a per-block empty (all-zero) chunk.
sparsity = NUM_USER*NUM_ITEM / #unique(row,col), host-computed from
indices only.
"""
import sys

import numpy as np

_NUM_USER, _NUM_ITEM, _H = 4096, 16384, 64
_N_CORES = 8
_P = 128
_U_SHARD = _NUM_USER // _N_CORES          # 512
_OUT_ELEMS = _U_SHARD * _NUM_ITEM         # 8388608
_N_BLK = _U_SHARD // _P                   # 4
_W = 64                                   # scatter chunk width (f32 elems)
_CHUNKS_PER_ROW = _NUM_ITEM // _W         # 256
_CHUNKS_PER_BLK = _P * _CHUNKS_PER_ROW    # 32768


def _ensure_concourse():
    try:
        import concourse  # noqa: F401
    except ImportError:
        for p in ("/opt/trn_rl_repo", "/root/.axon_site/_ro/trn_rl_repo"):
            if p not in sys.path:
                sys.path.insert(0, p)


def _build(opb, nfix):
    """opb: chunk-scatter ops per 128-row block; nfix: fixup ops."""
    from concourse import bass, mybir

    f32, i32 = mybir.dt.float32, mybir.dt.int32
    nops = _N_BLK * opb                   # chunk-scatter ops per output
    ncol = nops + nfix                    # dot columns (slots per partition)
    nc = bass.Bass()
    ug = nc.declare_dram_parameter("ug", [_P, ncol * _H], f32, isOutput=False)
    ig = nc.declare_dram_parameter("ig", [_P, ncol * _H], f32, isOutput=False)
    msk = nc.declare_dram_parameter("msk", [_P, nops * _W], f32, isOutput=False)
    lpay = nc.declare_dram_parameter("lpay", [_P, nops * _W], f32, isOutput=False)
    coffs = nc.declare_dram_parameter("coffs", [_P, nops], i32, isOutput=False)
    foffs = nc.declare_dram_parameter("foffs", [_P, max(nfix, 1)], i32, isOutput=False)
    pred = nc.declare_dram_parameter("pred", [_U_SHARD, _NUM_ITEM], f32, isOutput=True)
    label = nc.declare_dram_parameter("label", [_U_SHARD, _NUM_ITEM], f32, isOutput=True)

    with (
        nc.Block() as block,
        nc.semaphore("mset_sem") as mset_sem,
        nc.semaphore("load_sem") as load_sem,
        nc.semaphore("payl_sem") as payl_sem,
        nc.semaphore("zp_sem") as zp_sem,
        nc.semaphore("zl_sem") as zl_sem,
        nc.semaphore("sc_sem") as sc_sem,
        nc.sbuf_tensor([_P, _NUM_ITEM], f32) as zero_sb,
        nc.sbuf_tensor([_P, ncol * _H], f32) as ug_sb,
        nc.sbuf_tensor([_P, ncol * _H], f32) as ig_sb,
        nc.sbuf_tensor([_P, ncol * _H], f32) as prod_sb,
        nc.sbuf_tensor([_P, ncol], f32) as dots_sb,
        nc.sbuf_tensor([_P, nops * _W], f32) as msk_sb,
        nc.sbuf_tensor([_P, nops * _W], f32) as lpay_sb,
        nc.sbuf_tensor([_P, nops * _W], f32) as ppay_sb,
        nc.sbuf_tensor([_P, nops], i32) as coffs_sb,
        nc.sbuf_tensor([_P, max(nfix, 1)], i32) as foffs_sb,
    ):
        @block.vector
        def _(v: bass.BassEngine):
            v.memset(zero_sb[:], 0.0).then_inc(mset_sem, 1)
            v.wait_ge(load_sem, 96)
            v.tensor_tensor(
                out=prod_sb[:], in0=ug_sb[:], in1=ig_sb[:],
                op=mybir.AluOpType.mult,
            )
            v.tensor_reduce(
                out=dots_sb[:],
                in_=prod_sb[:].rearrange("p (s h) -> p s h", h=_H),
                axis=mybir.AxisListType.X,
                op=mybir.AluOpType.add,
            )
            v.tensor_tensor(
                out=ppay_sb[:],
                in0=dots_sb[:, :nops].unsqueeze(-1).to_broadcast([_P, nops, _W]),
                in1=msk_sb[:].rearrange("p (s w) -> p s w", w=_W),
                op=mybir.AluOpType.mult,
            ).then_inc(payl_sem, 1)

        @block.sync
        def _(s: bass.BassEngine):
            s.dma_start(out=coffs_sb[:], in_=coffs[:]).then_inc(load_sem, 16)
            s.dma_start(out=foffs_sb[:], in_=foffs[:]).then_inc(load_sem, 16)
            s.dma_start(out=lpay_sb[:], in_=lpay[:]).then_inc(load_sem, 16)
            s.dma_start(out=ug_sb[:], in_=ug[:]).then_inc(load_sem, 16)
            s.dma_start(out=ig_sb[:], in_=ig[:]).then_inc(load_sem, 16)
            s.dma_start(out=msk_sb[:], in_=msk[:]).then_inc(load_sem, 16)
            s.wait_ge(mset_sem, 1)
            for b in range(_N_BLK):
                s.dma_start(
                    out=label[b * _P:(b + 1) * _P, :], in_=zero_sb[:]
                ).then_inc(zl_sem, 16)
                s.dma_start(
                    out=pred[b * _P:(b + 1) * _P, :], in_=zero_sb[:]
                ).then_inc(zp_sem, 16)

        pred_chunks = pred[:, :].rearrange("a (c w) -> (a c) w", w=_W)
        label_chunks = label[:, :].rearrange("a (c w) -> (a c) w", w=_W)
        pred_flat = pred[:, :].rearrange("a b -> (a b)").unsqueeze(-1)

        @block.gpsimd
        def _(g: bass.BassEngine):
            g.wait_ge(load_sem, 96)
            n_sc = 0
            for b in range(_N_BLK):
                g.wait_ge(zl_sem, 16 * (b + 1))
                for j in range(opb):
                    col = b * opb + j
                    g.indirect_dma_start(
                        out=label_chunks,
                        out_offset=bass.IndirectOffsetOnAxis(
                            ap=coffs_sb[:, col:col + 1], axis=0
                        ),
                        in_=lpay_sb[:, col * _W:(col + 1) * _W],
                        in_offset=None,
                    ).then_inc(sc_sem, 16)
                    n_sc += 16
                if b == 0:
                    g.wait_ge(payl_sem, 1)
                g.wait_ge(zp_sem, 16 * (b + 1))
                for j in range(opb):
                    col = b * opb + j
                    g.indirect_dma_start(
                        out=pred_chunks,
                        out_offset=bass.IndirectOffsetOnAxis(
                            ap=coffs_sb[:, col:col + 1], axis=0
                        ),
                        in_=ppay_sb[:, col * _W:(col + 1) * _W],
                        in_offset=None,
                    ).then_inc(sc_sem, 16)
                    n_sc += 16
            for k in range(nfix):
                g.indirect_dma_start(
                    out=pred_flat,
                    out_offset=bass.IndirectOffsetOnAxis(
                        ap=foffs_sb[:, k:k + 1], axis=0
                    ),
                    in_=dots_sb[:, nops + k:nops + k + 1],
                    in_offset=None,
                ).then_inc(sc_sem, 16)
                n_sc += 16
            g.wait_ge(sc_sem, n_sc)

    return nc


def _route(idx_user, item_sets, rating_sets, embed_user, embed_item):
    rows = np.asarray(idx_user).astype(np.int64)
    cols = np.asarray(item_sets).astype(np.int64)
    vals = np.asarray(rating_sets).astype(np.float32)
    eu = np.asarray(embed_user, dtype=np.float32)
    ei = np.asarray(embed_item, dtype=np.float32)

    keys = (rows[:, None] * _NUM_ITEM + cols).ravel()
    fvals = vals.ravel()
    perm = np.argsort(keys, kind="stable")
    skeys = keys[perm]
    is_last = np.r_[skeys[1:] != skeys[:-1], True]
    ukeys = skeys[is_last]
    uvals = fvals[perm[is_last]]
    n_unique = len(ukeys)

    # Duplicate (row, col) pairs: the scatter's winner is backend-specific.
    # Probe the actual jax backend's .at[].set semantics by scattering the
    # flat entry index with the same shapes, and read back only the
    # duplicated positions.
    grp_start = np.nonzero(np.r_[True, skeys[1:] != skeys[:-1]])[0]
    grp_cnt = np.diff(np.r_[grp_start, len(skeys)])
    dup_keys = skeys[grp_start[grp_cnt > 1]]
    if len(dup_keys):
        import jax.numpy as jnp

        e_ids = jnp.arange(keys.size, dtype=jnp.float32).reshape(cols.shape)
        wmat = jnp.zeros((_NUM_USER, _NUM_ITEM), jnp.float32).at[
            jnp.asarray(rows)[:, None], jnp.asarray(cols)
        ].set(e_ids)
        dr, dc = dup_keys // _NUM_ITEM, dup_keys % _NUM_ITEM
        win_e = np.asarray(wmat[jnp.asarray(dr), jnp.asarray(dc)]).astype(np.int64)
        upos = np.searchsorted(ukeys, dup_keys)
        uvals[upos] = fvals[win_e]

    urow = ukeys // _NUM_ITEM
    ucol = ukeys % _NUM_ITEM
    core = urow // _U_SHARD
    lrow = urow - core * _U_SHARD
    blk = lrow // _P
    chunk = lrow * _CHUNKS_PER_ROW + ucol // _W     # [0, 131072) per core
    pos = ucol % _W
    eoff = lrow * _NUM_ITEM + ucol                  # flat element offset

    # group by (core, blk, chunk); entries already sorted by key => by all
    cb = core * _N_BLK + blk
    per = {}
    max_chunks = 1
    max_extra = 0
    for c in range(_N_CORES):
        m = core == c
        info = {"chunks": [], "extra": []}
        for b in range(_N_BLK):
            mb = m & (blk == b)
            ch_b = chunk[mb]
            first = np.r_[True, ch_b[1:] != ch_b[:-1]] if len(ch_b) else np.array([], bool)
            idxs = np.nonzero(mb)[0]
            info["chunks"].append((idxs[first] if len(ch_b) else idxs,
                                   idxs[~first] if len(ch_b) else idxs[:0]))
            if len(ch_b):
                max_chunks = max(max_chunks, int(first.sum()))
        per[c] = info
    opb = -(-max_chunks // _P)
    for c in range(_N_CORES):
        n_extra = sum(len(e) for _, e in per[c]["chunks"])
        max_extra = max(max_extra, n_extra)
    nfix = -(-max_extra // _P) if max_extra else 0

    nops = _N_BLK * opb
    ncol = nops + nfix
    in_maps = []
    for c in range(_N_CORES):
        slots_u = np.zeros((ncol * _P, _H), np.float32)   # slot-major [t, h]
        slots_i = np.zeros((ncol * _P, _H), np.float32)
        mask = np.zeros((nops * _P, _W), np.float32)
        lp = np.zeros((nops * _P, _W), np.float32)
        co = np.zeros((nops, _P), np.int32)
        fo = np.zeros((max(nfix, 1), _P), np.int32)

        extras_all = []
        empty_elem = None
        for b in range(_N_BLK):
            mains, extras = per[c]["chunks"][b]
            extras_all.append(extras)
            # sacrificial empty chunk for this block
            occ = set(chunk[mains].tolist())
            base = b * _CHUNKS_PER_BLK
            sac = next(x for x in range(base, base + _CHUNKS_PER_BLK)
                       if x not in occ)
            if empty_elem is None:
                empty_elem = sac * _W
            co[b * opb:(b + 1) * opb, :] = sac
            nm = len(mains)
            # slot t within block: op j = t // P, partition p = t % P
            t = np.arange(nm)
            op_global = b * opb + t // _P
            p = t % _P
            slot_flat = op_global * _P + p                # [0, nops*P)
            co[op_global, p] = chunk[mains]
            slots_u[slot_flat] = eu[urow[mains]]
            slots_i[slot_flat] = ei[ucol[mains]]
            mask[slot_flat, pos[mains]] = 1.0
            # label payload: all members of each chunk merged
            grp_all = np.nonzero((core == c) & (blk == b))[0]
            if len(grp_add := grp_all):
                ch_of = chunk[grp_add]
                # map each entry's chunk to its main slot via searchsorted
                mains_ch = chunk[mains]
                pos_in_mains = np.searchsorted(mains_ch, ch_of)
                sf = op_global[pos_in_mains] * _P + p[pos_in_mains]
                lp[sf, pos[grp_add]] = uvals[grp_add]

        extras_all = np.concatenate(extras_all) if extras_all else np.array([], np.int64)
        if empty_elem is None:
            empty_elem = 0
        fo[:, :] = empty_elem
        ne = len(extras_all)
        if ne:
            t = np.arange(ne)
            kf = t // _P
            pf = t % _P
            fo[kf, pf] = eoff[extras_all]
            slot_flat = (nops + kf) * _P + pf
            slots_u[slot_flat] = eu[urow[extras_all]]
            slots_i[slot_flat] = ei[ucol[extras_all]]

        def tocols(a, w):
            # [ncols*P, w] slot-major -> [P, ncols*w]
            return np.ascontiguousarray(
                a.reshape(-1, _P, w).transpose(1, 0, 2).reshape(_P, -1)
            )

        in_maps.append({
            "ug": tocols(slots_u, _H),
            "ig": tocols(slots_i, _H),
            "msk": tocols(mask, _W),
            "lpay": tocols(lp, _W),
            "coffs": np.ascontiguousarray(co.T),
            "foffs": np.ascontiguousarray(fo.T),
        })
    return in_maps, n_unique, opb, nfix


def kernel(idx_user, item_sets, rating_sets, embed_user, embed_item):
    _ensure_concourse()
    from concourse.bass_utils import run_bass_kernel_spmd

    in_maps, n_unique, opb, nfix = _route(
        idx_user, item_sets, rating_sets, embed_user, embed_item
    )
    nc = _build(opb, nfix)
    res = run_bass_kernel_spmd(nc, in_maps, list(range(_N_CORES))).results

    pred = np.empty((_NUM_USER, _NUM_ITEM), np.float32)
    label = np.empty((_NUM_USER, _NUM_ITEM), np.float32)
    for c in range(_N_CORES):
        pred[c * _U_SHARD:(c + 1) * _U_SHARD] = res[c]["pred"]
        label[c * _U_SHARD:(c + 1) * _U_SHARD] = res[c]["label"]
    sparsity = np.float32(np.float32(_NUM_USER * _NUM_ITEM) / np.float32(n_unique))
    return pred, label, sparsity
